# revision 1
# baseline (speedup 1.0000x reference)
"""AttnBlock3D (GroupNorm + single-head self-attention + proj + residual) on 8 trn2 cores.

Sharding: core i handles (batch b = i//4, query-block qb = i%4) of 1024 query
positions. Attention is permutation-equivariant over positions, so each core
receives its batch's x with the position axis rolled so that its query block
occupies columns 0:1024. Each core computes GroupNorm + full V for its batch
(4x replicated within a batch group) and attention/proj/residual for its own
1024 query positions. No collectives.

Algebraic restructures (exact up to fp rounding):
  * Q and K projections are never materialized. With Wqk = Wk^T Wq and
    bqk = Wk^T bq (host-computed),
      scores^T[nk, nq] = xn[:, nk] . (Wqk xn[:, :1024] + bqk)[:, nq]
                         + (per-nq constants, which cancel in softmax).
    One [512x512] @ [512x1024] matmul replaces both the Q and K projections.
  * Softmax skips the max subtraction (scores*scale ~ N(0,1); exp is safe in
    fp32) and the normalization is deferred past the output projection:
      out = x + proj(V exp(s)) * (1/rowsum) + (Wp bv + pb),
    so the rowsum -> reciprocal -> broadcast chain never gates a matmul.
    max |V exp(s)| ~ 2.5e3, safely inside fp16 range.
  * The rowsum is a ones-vector matmul on the exp tiles (cross-partition
    reduction), broadcast back across partitions by a rank-1 f32 matmul.

Layouts (per core, channel-major [c(partitions), n(free)] unless noted):
  x    [512, 4096] f32   (rolled input)
  xn   [512, 4096] f16   group-normalized
  b    [512, 1024] f16   B = Wqk @ xn_q + bqk
  vT   [4096, 512] f16   V^T (position-major), computed directly by matmul
  s^T  [nk, nq]          scores transposed -> exp (scaled) -> p^T f16
  o^T  [512, 1024] f16   V @ p^T, unnormalized
  out  [512, 1024] f32   x + proj(o)/rowsum + (Wp @ bv + pb)

GroupNorm: per-channel mean/var via bn_stats/bn_aggr as x pieces land, then a
cross-partition group reduce and per-channel broadcast via tiny matmuls with
host-built selection matrices. PE warm-up matmuls paced by the x DMA keep the
HAM activity monitor at 2.4 GHz through the load phase.

All matmuls use fp16 operands (1 cycle/row on the PE) with fp32 PSUM
accumulation.
"""

import numpy as np

import concourse.bass as bass
import concourse.tile as tile
from concourse import bacc, mybir
from concourse.bass import ds, ts
from concourse.bass_utils import run_bass_kernel_spmd

B, C, H, W, D = 2, 512, 16, 16, 16
N = H * W * D              # 4096 positions
NQ = N // 4                # 1024 query positions per core
T = C // 128               # 4 channel tiles
NKT = N // 128             # 32 key tiles
NQC = NQ // 512            # 2 query chunks of 512
NCH = N // 512             # 8 column chunks of 512
GROUPS = 32
GSIZE = C // GROUPS        # 16 channels per group
EPS = 1e-6
SCALE = float(C) ** -0.5

F32 = mybir.dt.float32
F16 = mybir.dt.float16


def build_nc(reps: int = 1):
    nc = bacc.Bacc("TRN2", target_bir_lowering=False)

    env = {}
    env["x_d"] = nc.dram_tensor("x", [C, N], F32, kind="ExternalInput")
    env["wqkT_d"] = nc.dram_tensor("wqkT", [C, C], F16, kind="ExternalInput")
    env["wvT_d"] = nc.dram_tensor("wvT", [C, C], F16, kind="ExternalInput")
    env["wpT_d"] = nc.dram_tensor("wpT", [C, C], F16, kind="ExternalInput")
    env["bqk_d"] = nc.dram_tensor("bqk", [128, T], F32, kind="ExternalInput")
    env["gnw_d"] = nc.dram_tensor("gnw", [128, T], F32, kind="ExternalInput")
    env["gnb_d"] = nc.dram_tensor("gnb", [128, T], F32, kind="ExternalInput")
    env["fb_d"] = nc.dram_tensor("fb", [128, T], F32, kind="ExternalInput")
    env["selred_d"] = nc.dram_tensor("selred", [128, T, GROUPS], F32, kind="ExternalInput")
    env["selbc_d"] = nc.dram_tensor("selbc", [GROUPS, C], F32, kind="ExternalInput")
    env["out_d"] = nc.dram_tensor("out", [C, NQ], F32, kind="ExternalOutput")

    with tile.TileContext(nc) as tc:
        import contextlib

        with contextlib.ExitStack() as ctx:
            env["const"] = ctx.enter_context(tc.tile_pool(name="const", bufs=1))
            env["big"] = ctx.enter_context(tc.tile_pool(name="big", bufs=1))
            env["mid"] = ctx.enter_context(tc.tile_pool(name="mid", bufs=1))
            env["stats"] = ctx.enter_context(tc.tile_pool(name="stats", bufs=2))
            env["small"] = ctx.enter_context(tc.tile_pool(name="small", bufs=2))
            env["ppool"] = ctx.enter_context(tc.tile_pool(name="ppool", bufs=16))
            env["ps_work"] = ctx.enter_context(tc.tile_pool(name="ps_work", bufs=2, space="PSUM"))
            env["ps_o"] = ctx.enter_context(tc.tile_pool(name="ps_o", bufs=4, space="PSUM"))
            env["ps_rs"] = ctx.enter_context(tc.tile_pool(name="ps_rs", bufs=1, space="PSUM"))
            env["ps_small"] = ctx.enter_context(tc.tile_pool(name="ps_small", bufs=1, space="PSUM"))

            const = env["const"]
            ones16 = const.tile([128, 1], F16, tag="ones16")
            nc.vector.memset(ones16, 1.0)
            env["ones16"] = ones16
            ones32 = const.tile([1, 128], F32, tag="ones32")
            nc.vector.memset(ones32, 1.0)
            env["ones32"] = ones32
            epst = const.tile([GROUPS, 1], F32, tag="epst")
            nc.vector.memset(epst, EPS)
            env["epst"] = epst
            ones32c = const.tile([128, 1], F32, tag="ones32c")
            nc.vector.memset(ones32c, 1.0)
            env["ones32c"] = ones32c

            for rep in range(reps):
                body(nc, tc, env, first=(rep == 0))

    nc.compile()
    return nc


def body(nc, tc, env, first=True):
    big, mid, stats, small, ppool = (env[k] for k in ("big", "mid", "stats", "small", "ppool"))
    ps_work, ps_o, ps_rs, ps_small = (env[k] for k in ("ps_work", "ps_o", "ps_rs", "ps_small"))
    x_d, out_d = env["x_d"], env["out_d"]
    const = env["const"]
    ones16, ones32, epst, ones32c = (env[k] for k in ("ones16", "ones32", "epst", "ones32c"))
    AF = mybir.ActivationFunctionType

    # -------- load x (first, it gates everything) + GroupNorm stats --------
    # x arrives in 1MB pieces ([128, 2048] = 8KB/partition lines); bn_stats
    # runs per 512-chunk as pieces land. Dummy warm-up matmuls paced by the
    # arriving pieces keep the PE's HAM clock at 2.4 GHz through the head.
    x_sb = big.tile([128, T, N], F32, tag="x")
    sts = []
    for t in range(T):
        st = stats.tile([128, NCH, 6], F32, tag=f"bnstats{t}", bufs=1, name=f"st{t}")
        sts.append(st)
    # PE warm-up strategy: the HAM clock gate needs a ~3.4us DENSE busy window
    # to un-throttle (sparse activity maintains but does not trigger). A dense
    # f32 dummy-matmul block paced by the first x piece triggers it while the
    # rest of x is still in flight; one dummy matmul per later piece keeps
    # every subsequent activity window non-idle until the real stream begins.
    for t in range(T):
        # alternate the two HWDGE issue engines for more DMA parallelism;
        # the LAST tile arrives in two pieces so its bn_stats (which gate the
        # whole GroupNorm chain and thus the entire matmul stream) overlap
        # the tail of the DMA instead of serializing after it
        eng = nc.sync if t % 2 == 0 else nc.scalar
        pieces = 1 if t < T - 1 else 4
        for piece in range(pieces):
            w = N // pieces
            eng.dma_start(out=x_sb[:, t, ds(piece * w, w)],
                          in_=x_d[ts(t, 128), ds(piece * w, w)])
            for s in range(piece * (NCH // pieces), (piece + 1) * (NCH // pieces)):
                nc.vector.bn_stats(out=sts[t][:, s, :],
                                   in_=x_sb[:, t, ds(s * 512, 512)])
            n_wu = (14 if t < T - 1 else 4) // pieces * pieces // pieces
            for wu in range(n_wu):
                wu_ps = ps_rs.tile([1, 256], F32, tag="psrs",
                                   name=f"wu{t}_{piece}_{wu}")
                nc.tensor.matmul(wu_ps, ones32c,
                                 x_sb[:, t, ds(piece * w + (wu % (w // 256)) * 256, 256)],
                                 start=True, stop=True)

    # -------- constants (after x in DMA priority; loaded once) --------
    if first:
        for nm in ("wqkT", "wvT", "wpT"):
            sb = const.tile([128, T, C], F16, tag=nm, name=f"sb_{nm}")
            dr = env[f"{nm}_d"]
            for t in range(T):
                nc.sync.dma_start(out=sb[:, t, :], in_=dr[ts(t, 128), :])
            env[nm] = sb
        for nm in ("bqk", "gnw", "gnb", "fb"):
            sb = const.tile([128, T], F32, tag=nm, name=f"sb_{nm}")
            nc.sync.dma_start(out=sb, in_=env[f"{nm}_d"][:, :])
            env[nm] = sb
        selred = const.tile([128, T, GROUPS], F32, tag="selred")
        nc.sync.dma_start(out=selred, in_=env["selred_d"][:, :, :])
        env["selred"] = selred
        selbc = const.tile([GROUPS, C], F32, tag="selbc")
        nc.sync.dma_start(out=selbc, in_=env["selbc_d"][:, :])
        env["selbc"] = selbc
    wqkT, wvT, wpT = env["wqkT"], env["wvT"], env["wpT"]
    bqk, gnw, gnb, fb = env["bqk"], env["gnw"], env["gnb"], env["fb"]
    selred, selbc = env["selred"], env["selbc"]


    # -------- finish GroupNorm statistics --------
    mvs = []
    for t in range(T):
        mv = stats.tile([128, 2], F32, tag=f"mv{t}", bufs=1, name=f"mv{t}")
        nc.vector.bn_aggr(out=mv, in_=sts[t])
        # mv := (mean, E[x^2]) ; E[x^2] = var + mean^2
        msq = stats.tile([128, 1], F32, tag="msq")
        nc.vector.tensor_mul(msq, mv[:, 0:1], mv[:, 0:1])
        nc.vector.tensor_add(mv[:, 1:2], mv[:, 1:2], msq)
        mvs.append(mv)

    psg = ps_small.tile([GROUPS, 2], F32, tag="pssmall")
    for t in range(T):
        nc.tensor.matmul(psg, selred[:, t, :], mvs[t], start=(t == 0), stop=(t == T - 1))

    # group scale/offset: rstd = 1/sqrt(var+eps), offset = -mean*rstd
    psgs = small.tile([GROUPS, 2], F32, tag="psgs", bufs=1)
    nc.vector.tensor_copy(psgs, psg)
    gsc = small.tile([GROUPS, 2], F32, tag="gsc", bufs=1)
    gtmp = small.tile([GROUPS, 2], F32, tag="gtmp", bufs=1)
    nc.vector.tensor_mul(gtmp[:, 0:1], psgs[:, 0:1], psgs[:, 0:1])      # mean^2
    nc.vector.tensor_sub(gtmp[:, 1:2], psgs[:, 1:2], gtmp[:, 0:1])      # var
    nc.scalar.activation(out=gsc[:, 0:1], in_=gtmp[:, 1:2], func=AF.Sqrt, bias=epst)
    nc.vector.reciprocal(gsc[:, 0:1], gsc[:, 0:1])                      # rstd
    nc.vector.tensor_mul(gsc[:, 1:2], psgs[:, 0:1], gsc[:, 0:1])       # mean*rstd
    nc.vector.tensor_scalar_mul(gsc[:, 1:2], gsc[:, 1:2], -1.0)        # offset

    # broadcast to per-channel scale/offset, fold gn weight/bias
    scof = small.tile([128, T, 2], F32, tag="scof", bufs=1)
    for t in range(T):
        psbc = ps_small.tile([128, 2], F32, tag="pssmall")
        nc.tensor.matmul(psbc, selbc[:, ts(t, 128)], gsc, start=True, stop=True)
        nc.vector.tensor_mul(scof[:, t, 0:1], psbc[:, 0:1], gnw[:, t:t + 1])
        nc.vector.tensor_mul(scof[:, t, 1:2], psbc[:, 1:2], gnw[:, t:t + 1])
        nc.vector.tensor_add(scof[:, t, 1:2], scof[:, t, 1:2], gnb[:, t:t + 1])

    # -------- apply GN -> xn (f16), n-chunked so consumers pipeline --------
    xn = mid.tile([128, T, N], F16, tag="xn")
    for nch in range(NCH):
        for t in range(T):
            nc.vector.tensor_scalar(
                out=xn[:, t, ds(nch * 512, 512)], in0=x_sb[:, t, ds(nch * 512, 512)],
                scalar1=scof[:, t, 0:1], scalar2=scof[:, t, 1:2],
                op0=mybir.AluOpType.mult, op1=mybir.AluOpType.add,
            )

    # -------- B = Wqk xn_q + bqk --------
    b_sb = mid.tile([128, T, NQ], F16, tag="b")
    for t_out in range(T):
        for nch in range(NQC):
            ps = ps_work.tile([128, 512], F32, tag="pswork")
            for tc_in in range(T):
                nc.tensor.matmul(ps, wqkT[:, tc_in, ts(t_out, 128)],
                                 xn[:, tc_in, ds(nch * 512, 512)],
                                 start=(tc_in == 0), stop=(tc_in == T - 1))
            nc.scalar.activation(out=b_sb[:, t_out, ds(nch * 512, 512)], in_=ps,
                                 func=AF.Identity, bias=bqk[:, t_out:t_out + 1])

    # -------- V^T --------
    vT = big.tile([128, NKT, C], F16, tag="vT")
    for nkt in range(NKT):
        ps = ps_work.tile([128, 512], F32, tag="pswork")
        for tc_in in range(T):
            nc.tensor.matmul(ps, xn[:, tc_in, ts(nkt, 128)], wvT[:, tc_in, :],
                             start=(tc_in == 0), stop=(tc_in == T - 1))
        nc.scalar.activation(out=vT[:, nkt, :], in_=ps, func=AF.Identity, bias=0.0)

    # residual slice + folded bias, loaded late (off the head's DMA critical
    # path; only needed by the proj/residual stage)
    xq = mid.tile([128, T, NQ], F32, tag="xq")
    for t in range(T):
        nc.sync.dma_start(out=xq[:, t, :], in_=x_d[ts(t, 128), 0:NQ])
        nc.vector.tensor_scalar_add(xq[:, t, :], xq[:, t, :], fb[:, t:t + 1])

    # -------- attention + proj per query chunk --------
    # The PE-side epilogue of chunk ch (rowsum-broadcast matmul + proj) is
    # deferred into the middle of chunk ch+1's k-loop: by then the reciprocal
    # is long done, so the PE never stalls on the normalization chain, and no
    # >2us PE gap opens at the chunk boundary (which would trip the HAM
    # clock gate into its half-rate state).
    def pe_epilogue(ch):
        bc_ps = ps_small.tile([128, 512], F32, tag="pssmall", name=f"bcps{ch}")
        nc.tensor.matmul(bc_ps, ones32, env[f"rsinv{ch}"], start=True, stop=True)
        bc_sb = small.tile([128, 512], F32, tag="bc", name=f"bcsb{ch}")
        nc.vector.tensor_copy(bc_sb, bc_ps)
        for t_out in range(T):
            # chunk 0: head bank (keeps ps_work free for chunk 1's scores);
            # final chunk: rotate 3 slots (2x ps_work + head bank) so its four
            # accumulation groups don't serialize against the epilogue reads
            if ch == 0 or t_out == 0:
                ps = ps_small.tile([128, 512], F32, tag="pssmall", name=f"prps{ch}_{t_out}")
            else:
                ps = ps_work.tile([128, 512], F32, tag="pswork", name=f"prps{ch}_{t_out}")
            for tc_in in range(T):
                nc.tensor.matmul(ps, wpT[:, tc_in, ts(t_out, 128)],
                                 o_sb[:, tc_in, ds(ch * 512, 512)],
                                 start=(tc_in == 0), stop=(tc_in == T - 1))
            pn = small.tile([128, 512], F32, tag="pn", name=f"pn{ch}_{t_out}")
            nc.vector.tensor_mul(pn, ps, bc_sb)
            nc.vector.tensor_add(xq[:, t_out, ds(ch * 512, 512)],
                                 xq[:, t_out, ds(ch * 512, 512)], pn)
            nc.sync.dma_start(out=out_d[ts(t_out, 128), ds(ch * 512, 512)],
                              in_=xq[:, t_out, ds(ch * 512, 512)])

    o_sb = mid.tile([128, T, NQ], F16, tag="o")
    for ch in range(NQC):
        o_ps = [ps_o.tile([128, 512], F32, tag="pso", name=f"ops{ch}_{i}")
                for i in range(T)]
        rs_ps = ps_rs.tile([1, 512], F32, tag="psrs")
        # Software-pipelined by one stage: tile i+1's score matmuls are
        # emitted between tile i's scores and tile i's AV so the PE has work
        # while the exp (ScalarE) for tile i is still in flight.
        def emit_av(nkt, p_t):
            # rowsum first: its 1-column LDWEIGHTS is nearly free and fills
            # the pipeline while the exp->AV semaphore settles
            nc.tensor.matmul(rs_ps, ones16, p_t,
                             start=(nkt == 0), stop=(nkt == NKT - 1))
            for tc_in in range(T):
                nc.tensor.matmul(o_ps[tc_in], vT[:, nkt, ts(tc_in, 128)], p_t,
                                 start=(nkt == 0), stop=(nkt == NKT - 1))

        prev = None
        for nkt in range(NKT):
            s_ps = ps_work.tile([128, 512], F32, tag="pswork")
            for tc_in in range(T):
                nc.tensor.matmul(s_ps, xn[:, tc_in, ts(nkt, 128)],
                                 b_sb[:, tc_in, ds(ch * 512, 512)],
                                 start=(tc_in == 0), stop=(tc_in == T - 1))
            p_t = ppool.tile([128, 512], F16, tag="p")
            nc.scalar.activation(out=p_t, in_=s_ps, func=AF.Exp, scale=SCALE)
            if prev is not None:
                emit_av(prev[0], prev[1])
            prev = (nkt, p_t)
            if ch > 0 and nkt == 6:
                pe_epilogue(ch - 1)
        emit_av(prev[0], prev[1])

        # rowsum reciprocal (DVE) + unnormalized-o eviction (ScalarE) happen
        # immediately — the eviction frees the o accumulators for the next
        # chunk's AV matmuls. |o| < ~3e3, safely fp16.
        rsinv = small.tile([1, 512], F32, tag="rsinv", name=f"rsinv{ch}")
        nc.vector.reciprocal(rsinv, rs_ps)
        env[f"rsinv{ch}"] = rsinv
        # evictions split across ScalarE and VectorE to halve the latency
        # before the next chunk's AV matmuls can claim the o accumulators
        for tc_in in range(T):
            if tc_in % 2 == 0:
                nc.scalar.activation(out=o_sb[:, tc_in, ds(ch * 512, 512)],
                                     in_=o_ps[tc_in], func=AF.Identity, bias=0.0)
            else:
                nc.vector.tensor_copy(o_sb[:, tc_in, ds(ch * 512, 512)],
                                      o_ps[tc_in])

    pe_epilogue(NQC - 1)


_NC_CACHE = {}


def _get_nc(reps: int = 1):
    if reps not in _NC_CACHE:
        _NC_CACHE[reps] = build_nc(reps)
    return _NC_CACHE[reps]


def make_in_maps(x, gn_weight, gn_bias, qkv_weight, qkv_bias, proj_weight, proj_bias):
    x = np.asarray(x, np.float32)
    qkv_weight = np.asarray(qkv_weight, np.float32)
    proj_weight = np.asarray(proj_weight, np.float32)
    qkv_bias = np.asarray(qkv_bias, np.float32)
    proj_bias = np.asarray(proj_bias, np.float32)
    gn_weight = np.asarray(gn_weight, np.float32)
    gn_bias = np.asarray(gn_bias, np.float32)

    Wq, Wk, Wv = qkv_weight[0:C], qkv_weight[C:2 * C], qkv_weight[2 * C:3 * C]
    wqkT = np.ascontiguousarray((Wq.T @ Wk).astype(np.float16))
    wvT = np.ascontiguousarray(Wv.T.astype(np.float16))
    wpT = np.ascontiguousarray(proj_weight.T.astype(np.float16))

    def cols(v):  # [C] -> [128, T]
        return np.ascontiguousarray(v.reshape(T, 128).T.astype(np.float32))

    bqkv = Wk.T @ qkv_bias[0:C]
    fbv = proj_weight @ qkv_bias[2 * C:3 * C] + proj_bias

    p_idx = np.arange(128)
    selred = np.zeros((128, T, GROUPS), np.float32)
    selbc = np.zeros((GROUPS, C), np.float32)
    for t in range(T):
        g = t * (128 // GSIZE) + p_idx // GSIZE
        selred[p_idx, t, g] = 1.0 / GSIZE
        selbc[g, t * 128 + p_idx] = 1.0

    shared = {
        "wqkT": wqkT, "wvT": wvT, "wpT": wpT,
        "bqk": cols(bqkv),
        "gnw": cols(gn_weight), "gnb": cols(gn_bias), "fb": cols(fbv),
        "selred": selred, "selbc": selbc,
    }
    in_maps = []
    for core in range(8):
        b, qb = core // 4, core % 4
        xb = x[b].reshape(C, N)
        xr = np.ascontiguousarray(np.roll(xb, -qb * NQ, axis=1))
        m = dict(shared)
        m["x"] = xr
        in_maps.append(m)
    return in_maps


def kernel(x, gn_weight, gn_bias, qkv_weight, qkv_bias, proj_weight, proj_bias):
    nc = _get_nc(1)
    in_maps = make_in_maps(x, gn_weight, gn_bias, qkv_weight, qkv_bias,
                           proj_weight, proj_bias)
    res = run_bass_kernel_spmd(nc, in_maps, core_ids=list(range(8)))
    out = np.empty((B, C, N), np.float32)
    for core in range(8):
        b, qb = core // 4, core % 4
        out[b][:, qb * NQ:(qb + 1) * NQ] = res.results[core]["out"]
    return out.reshape(B, C, H, W, D)



# revision 2
# speedup vs baseline: 1.9225x; 1.9225x over previous
"""AttnBlock3D (GroupNorm + single-head self-attention + proj + residual) on 8 trn2 cores.

Sharding: core i handles (batch b = i//4, query-block qb = i%4) of 1024 query
positions. Attention is permutation-equivariant over positions, so each core
receives its batch's x with the position axis rolled so that its query block
occupies columns 0:1024. Each core computes GroupNorm + full V for its batch
(4x replicated within a batch group) and attention/proj/residual for its own
1024 query positions. No collectives.

Algebraic restructures (exact up to fp rounding):
  * Q and K projections are never materialized. With Wqk = Wq^T Wk and
    bqk = Wk^T bq (host-computed),
      scores^T[nk, nq] = xn[:, nk] . (Wqk xn[:, :1024] + bqk)[:, nq]
                         + (per-nq constants, which cancel in softmax).
  * Softmax skips the max subtraction and the normalization is deferred past
    the output projection:
      out = x + proj(V exp(s - SH)) * (1/rowsum) + (Wp bv + pb),
    where the constant shift SH cancels between numerator and rowsum.
  * The rowsum is a ones-vector matmul on the exp tiles (cross-partition
    reduction), broadcast back across partitions by a rank-1 f32 matmul.

All big matmuls run in fp8 (e4m3) with perf_mode=DoubleRow: operands are
[128, 2, F] slices whose middle dim is a pair of 128-contraction tiles, giving
a 256-deep contraction per PE pass (2x fp16 throughput). Power-of-2 rescalings
keep every fp8 tensor inside TRN e4m3's +-240 range:
  * wqkT/wvT/wpT are host-scaled by WS=16 (escapes e4m3 subnormals),
  * exp is shifted by -SH=3 (max p ~ e^2.8), cancels in softmax,
  * V^T is evicted with scale 1/16 (undoes WS), o = V exp(s) evicted at
    scale 1, proj output multiplied by rsinv/16 (undoes wpT's WS).

x arrives as fp8 (host-cast, 2MB/core) in chunk-major layout
[128, chunk, tile, 512] so each chunk DMA is a 2KB-per-partition-line linear
transfer; the residual slice arrives separately as f16. GroupNorm stats via
bn_stats/bn_aggr per chunk as the DMA lands; group reduce and per-channel
broadcast via tiny host-built selection matmuls. The GN apply (x8 -> xn8)
runs on GpSimd (SBUF->SBUF), keeping DVE free for stats and evictions.
PE warm-up matmuls paced by the x DMA and by the bn_stats outputs keep the
HAM activity monitor at 2.4 GHz through the head.
"""

import numpy as np
import ml_dtypes

import concourse.bass as bass
import concourse.tile as tile
from concourse import bacc, mybir
from concourse.bass import ds, ts
from concourse.bass_utils import run_bass_kernel_spmd

B, C, H, W, D = 2, 512, 16, 16, 16
N = H * W * D              # 4096 positions
NQ = N // 4                # 1024 query positions per core
T = C // 128               # 4 channel tiles
NKT = N // 128             # 32 key tiles
NBL = NKT // 2             # 16 double key blocks (256 keys)
NQC = NQ // 512            # 2 query chunks of 512
NCH = N // 512             # 8 column chunks of 512
GROUPS = 32
GSIZE = C // GROUPS        # 16 channels per group
EPS = 1e-6
SCALE = float(C) ** -0.5
WS = 16.0                  # host-side weight scale (fp8 subnormal escape)
SH = 3.0                   # exp shift, cancels in softmax

F32 = mybir.dt.float32
F16 = mybir.dt.float16
F8 = mybir.dt.float8e4
DR = mybir.MatmulPerfMode.DoubleRow
E4 = ml_dtypes.float8_e4m3
AF = mybir.ActivationFunctionType


def build_nc(reps: int = 1):
    nc = bacc.Bacc("TRN2", target_bir_lowering=False)

    env = {}
    env["x8_d"] = nc.dram_tensor("x8", [128, NCH, T, 512], F8, kind="ExternalInput")
    env["xq_d"] = nc.dram_tensor("xq", [C, NQ], F16, kind="ExternalInput")
    env["wqkT_d"] = nc.dram_tensor("wqkT", [C, C], F8, kind="ExternalInput")
    env["wvT_d"] = nc.dram_tensor("wvT", [C, C], F8, kind="ExternalInput")
    env["wpT_d"] = nc.dram_tensor("wpT", [C, C], F8, kind="ExternalInput")
    env["bqk_d"] = nc.dram_tensor("bqk", [128, T], F32, kind="ExternalInput")
    env["gnw_d"] = nc.dram_tensor("gnw", [128, T], F32, kind="ExternalInput")
    env["gnb_d"] = nc.dram_tensor("gnb", [128, T], F32, kind="ExternalInput")
    env["fb_d"] = nc.dram_tensor("fb", [128, T], F32, kind="ExternalInput")
    env["selred_d"] = nc.dram_tensor("selred", [128, T, GROUPS], F32, kind="ExternalInput")
    env["selbc_d"] = nc.dram_tensor("selbc", [GROUPS, C], F32, kind="ExternalInput")
    env["out_d"] = nc.dram_tensor("out", [C, NQ], F32, kind="ExternalOutput")

    with tile.TileContext(nc) as tc:
        import contextlib

        with contextlib.ExitStack() as ctx:
            env["const"] = ctx.enter_context(tc.tile_pool(name="const", bufs=1))
            env["big"] = ctx.enter_context(tc.tile_pool(name="big", bufs=1))
            env["mid"] = ctx.enter_context(tc.tile_pool(name="mid", bufs=1))
            env["stats"] = ctx.enter_context(tc.tile_pool(name="stats", bufs=2))
            env["small"] = ctx.enter_context(tc.tile_pool(name="small", bufs=2))
            env["ppool"] = ctx.enter_context(tc.tile_pool(name="ppool", bufs=8))
            env["outp"] = ctx.enter_context(tc.tile_pool(name="outp", bufs=4))
            env["ps_s"] = ctx.enter_context(tc.tile_pool(name="ps_s", bufs=2, space="PSUM"))
            env["ps_o"] = ctx.enter_context(tc.tile_pool(name="ps_o", bufs=4, space="PSUM"))
            env["ps_rs"] = ctx.enter_context(tc.tile_pool(name="ps_rs", bufs=1, space="PSUM"))
            env["ps_x"] = ctx.enter_context(tc.tile_pool(name="ps_x", bufs=1, space="PSUM"))

            const = env["const"]
            ones8c = const.tile([128, 1], F8, tag="ones8c")
            nc.vector.memset(ones8c, 1.0)
            env["ones8c"] = ones8c
            ones8dr = const.tile([128, 2, 16], F8, tag="ones8dr")
            nc.vector.memset(ones8dr, 1.0)
            env["ones8dr"] = ones8dr
            ones32 = const.tile([1, 128], F32, tag="ones32")
            nc.vector.memset(ones32, 1.0)
            env["ones32"] = ones32
            ones32c = const.tile([128, 1], F32, tag="ones32c")
            nc.vector.memset(ones32c, 1.0)
            env["ones32c"] = ones32c
            epst = const.tile([GROUPS, 1], F32, tag="epst")
            nc.vector.memset(epst, EPS)
            env["epst"] = epst
            shb = const.tile([128, 1], F32, tag="shb")
            nc.vector.memset(shb, -SH)
            env["shb"] = shb

            for rep in range(reps):
                body(nc, tc, env, first=(rep == 0))

    nc.compile()
    return nc


def body(nc, tc, env, first=True):
    big, mid, stats, small, ppool, outp = (
        env[k] for k in ("big", "mid", "stats", "small", "ppool", "outp"))
    ps_s, ps_o, ps_rs, ps_x = (env[k] for k in ("ps_s", "ps_o", "ps_rs", "ps_x"))
    x8_d, out_d = env["x8_d"], env["out_d"]
    const = env["const"]
    ones8c, ones8dr, ones32, ones32c, epst, shb = (
        env[k] for k in ("ones8c", "ones8dr", "ones32", "ones32c", "epst", "shb"))

    # -------- load x8 (chunk-major) + GroupNorm stats as chunks land --------
    x8_sb = big.tile([128, NCH, T, 512], F8, tag="x8")
    sts = []
    for t in range(T):
        st = stats.tile([128, NCH, 6], F32, tag=f"bnstats{t}", bufs=1, name=f"st{t}")
        sts.append(st)
    # HAM warm-up: a dense fp8 dummy-matmul block paced by chunk 0 triggers the
    # 2.4 GHz state during the load; later, one tiny f32 matmul per bn_stats
    # output keeps every activity window non-idle until the real stream begins.
    for nch in range(NCH):
        nc.sync.dma_start(out=x8_sb[:, nch, :, :], in_=x8_d[:, nch, :, :])
        for t in range(T):
            nc.vector.bn_stats(out=sts[t][:, nch, :], in_=x8_sb[:, nch, t, :])
        if nch == 0:
            for wu in range(8):
                wu_ps = ps_rs.tile([1, 512], F32, tag="psrs", name=f"wu{nch}_{wu}")
                nc.tensor.matmul(wu_ps, ones8c, x8_sb[:, nch, wu % T, :],
                                 start=True, stop=True)
        else:
            for t in range(T):
                wu_ps = ps_rs.tile([1, 6], F32, tag="psrs", name=f"wus{nch}_{t}")
                nc.tensor.matmul(wu_ps, ones32c, sts[t][:, nch, :],
                                 start=True, stop=True)

    # -------- constants (after x in DMA priority; loaded once) --------
    if first:
        for nm in ("wqkT", "wvT", "wpT"):
            sb = const.tile([128, T, C], F8, tag=nm, name=f"sb_{nm}")
            dr = env[f"{nm}_d"]
            for t in range(T):
                nc.scalar.dma_start(out=sb[:, t, :], in_=dr[ts(t, 128), :])
            env[nm] = sb
        xq16 = const.tile([128, T, NQ], F16, tag="xq16")
        for t in range(T):
            nc.scalar.dma_start(out=xq16[:, t, :], in_=env["xq_d"][ts(t, 128), :])
        env["xq16"] = xq16
        for nm in ("bqk", "gnw", "gnb", "fb"):
            sb = const.tile([128, T], F32, tag=nm, name=f"sb_{nm}")
            nc.scalar.dma_start(out=sb, in_=env[f"{nm}_d"][:, :])
            env[nm] = sb
        selred = const.tile([128, T, GROUPS], F32, tag="selred")
        nc.scalar.dma_start(out=selred, in_=env["selred_d"][:, :, :])
        env["selred"] = selred
        selbc = const.tile([GROUPS, C], F32, tag="selbc")
        nc.scalar.dma_start(out=selbc, in_=env["selbc_d"][:, :])
        env["selbc"] = selbc
    wqkT, wvT, wpT = env["wqkT"], env["wvT"], env["wpT"]
    bqk, gnw, gnb, fb = env["bqk"], env["gnw"], env["gnb"], env["fb"]
    selred, selbc = env["selred"], env["selbc"]
    xq16 = env["xq16"]

    # -------- finish GroupNorm statistics --------
    mvs = []
    for t in range(T):
        mv = stats.tile([128, 2], F32, tag=f"mv{t}", bufs=1, name=f"mv{t}")
        nc.vector.bn_aggr(out=mv, in_=sts[t])
        # mv := (mean, E[x^2]) ; E[x^2] = var + mean^2
        msq = stats.tile([128, 1], F32, tag="msq")
        nc.vector.tensor_mul(msq, mv[:, 0:1], mv[:, 0:1])
        nc.vector.tensor_add(mv[:, 1:2], mv[:, 1:2], msq)
        mvs.append(mv)

    psg = ps_x.tile([GROUPS, 2], F32, tag="psx", name="psg")
    for t in range(T):
        nc.tensor.matmul(psg, selred[:, t, :], mvs[t], start=(t == 0), stop=(t == T - 1))

    # group scale/offset: rstd = 1/sqrt(var+eps), offset = -mean*rstd
    psgs = small.tile([GROUPS, 2], F32, tag="psgs", bufs=1)
    nc.vector.tensor_copy(psgs, psg)
    gsc = small.tile([GROUPS, 2], F32, tag="gsc", bufs=1)
    gtmp = small.tile([GROUPS, 2], F32, tag="gtmp", bufs=1)
    nc.vector.tensor_mul(gtmp[:, 0:1], psgs[:, 0:1], psgs[:, 0:1])      # mean^2
    nc.vector.tensor_sub(gtmp[:, 1:2], psgs[:, 1:2], gtmp[:, 0:1])      # var
    nc.scalar.activation(out=gsc[:, 0:1], in_=gtmp[:, 1:2], func=AF.Sqrt, bias=epst)
    nc.vector.reciprocal(gsc[:, 0:1], gsc[:, 0:1])                      # rstd
    nc.vector.tensor_mul(gsc[:, 1:2], psgs[:, 0:1], gsc[:, 0:1])       # mean*rstd
    nc.vector.tensor_scalar_mul(gsc[:, 1:2], gsc[:, 1:2], -1.0)        # offset

    # broadcast to per-channel scale/offset, fold gn weight/bias
    scof = small.tile([128, T, 2], F32, tag="scof", bufs=1)
    for t in range(T):
        psbc = ps_x.tile([128, 2], F32, tag="psx", name=f"psbc{t}")
        nc.tensor.matmul(psbc, selbc[:, ts(t, 128)], gsc, start=True, stop=True)
        nc.vector.tensor_mul(scof[:, t, 0:1], psbc[:, 0:1], gnw[:, t:t + 1])
        nc.vector.tensor_mul(scof[:, t, 1:2], psbc[:, 1:2], gnw[:, t:t + 1])
        nc.vector.tensor_add(scof[:, t, 1:2], scof[:, t, 1:2], gnb[:, t:t + 1])

    # -------- apply GN -> xn8 (fp8, on GpSimd), n-chunked so consumers pipeline --------
    xn8 = mid.tile([128, T, N], F8, tag="xn8")
    for nch in range(NCH):
        for t in range(T):
            nc.gpsimd.tensor_scalar(
                out=xn8[:, t, ds(nch * 512, 512)], in0=x8_sb[:, nch, t, :],
                scalar1=scof[:, t, 0:1], scalar2=scof[:, t, 1:2],
                op0=mybir.AluOpType.mult, op1=mybir.AluOpType.add,
            )

    # -------- B = WS*(Wqk xn_q + bqk), fp8 --------
    b8 = mid.tile([128, T, NQ], F8, tag="b8")
    for t_out in range(T):
        for qc in range(NQC):
            ps = ps_s.tile([128, 512], F32, tag="s")
            for j in range(T // 2):
                nc.tensor.matmul(ps, wqkT[:, 2 * j:2 * j + 2, ts(t_out, 128)],
                                 xn8[:, 2 * j:2 * j + 2, ds(qc * 512, 512)],
                                 start=(j == 0), stop=(j == T // 2 - 1), perf_mode=DR)
            nc.scalar.activation(out=b8[:, t_out, ds(qc * 512, 512)], in_=ps,
                                 func=AF.Identity, bias=bqk[:, t_out:t_out + 1])

    # -------- V^T (fp8; psum = WS*V, evicted at 1/WS) --------
    vT8 = big.tile([128, NKT, C], F8, tag="vT8")
    for nkt in range(NKT):
        ps = ps_s.tile([128, 512], F32, tag="s")
        for j in range(T // 2):
            nc.tensor.matmul(ps, xn8[:, 2 * j:2 * j + 2, ts(nkt, 128)],
                             wvT[:, 2 * j:2 * j + 2, :],
                             start=(j == 0), stop=(j == T // 2 - 1), perf_mode=DR)
        if nkt % 2 == 0:
            nc.scalar.activation(out=vT8[:, nkt, :], in_=ps,
                                 func=AF.Identity, scale=1.0 / WS)
        else:
            nc.vector.tensor_scalar_mul(vT8[:, nkt, :], ps, 1.0 / WS)

    # -------- attention + proj per query chunk --------
    # The PE-side epilogue of chunk ch (rowsum-broadcast matmul + proj) is
    # deferred into the middle of chunk ch+1's k-loop so the PE never stalls
    # on the normalization chain.
    o8 = mid.tile([128, T, NQ], F8, tag="o8")

    def pe_epilogue(ch):
        bc_ps = ps_x.tile([128, 512], F32, tag="psx", name=f"bcps{ch}")
        nc.tensor.matmul(bc_ps, ones32, env[f"rsinv{ch}"], start=True, stop=True)
        bc_sb = small.tile([128, 512], F32, tag="bc", name=f"bcsb{ch}")
        nc.vector.tensor_scalar_mul(bc_sb, bc_ps, 1.0 / WS)
        for t_out in range(T):
            if t_out == 0:
                ps = ps_x.tile([128, 512], F32, tag="psx", name=f"prps{ch}_{t_out}")
            else:
                ps = ps_s.tile([128, 512], F32, tag="s", name=f"prps{ch}_{t_out}")
            for j in range(T // 2):
                nc.tensor.matmul(ps, wpT[:, 2 * j:2 * j + 2, ts(t_out, 128)],
                                 o8[:, 2 * j:2 * j + 2, ds(ch * 512, 512)],
                                 start=(j == 0), stop=(j == T // 2 - 1), perf_mode=DR)
            pn = small.tile([128, 512], F32, tag="pn", name=f"pn{ch}_{t_out}")
            nc.vector.tensor_mul(pn, ps, bc_sb)
            ot = outp.tile([128, 512], F32, tag="ot", name=f"ot{ch}_{t_out}")
            nc.vector.scalar_tensor_tensor(
                out=ot, in0=xq16[:, t_out, ds(ch * 512, 512)],
                scalar=fb[:, t_out:t_out + 1], in1=pn,
                op0=mybir.AluOpType.add, op1=mybir.AluOpType.add)
            nc.sync.dma_start(out=out_d[ts(t_out, 128), ds(ch * 512, 512)], in_=ot)

    for ch in range(NQC):
        o_ps = [ps_o.tile([128, 512], F32, tag="o", name=f"ops{ch}_{i}")
                for i in range(T)]
        rs_ps = ps_rs.tile([1, 512], F32, tag="psrs", name=f"rs{ch}")

        # Software-pipelined by one stage: block i+1's score matmuls are
        # emitted between block i's scores and block i's AV so the PE has work
        # while the exp (ScalarE) for block i is still in flight.
        def emit_av(blk, p_t):
            nc.tensor.matmul(rs_ps, ones8dr[:, :, 0:1], p_t,
                             start=(blk == 0), stop=(blk == NBL - 1), perf_mode=DR)
            for tc_in in range(T):
                nc.tensor.matmul(o_ps[tc_in], vT8[:, 2 * blk:2 * blk + 2, ts(tc_in, 128)],
                                 p_t, start=(blk == 0), stop=(blk == NBL - 1),
                                 perf_mode=DR)

        prev = None
        for blk in range(NBL):
            p_t = ppool.tile([128, 2, 512], F8, tag="p")
            for half in range(2):
                s_ps = ps_s.tile([128, 512], F32, tag="s")
                for j in range(T // 2):
                    nc.tensor.matmul(s_ps, xn8[:, 2 * j:2 * j + 2, ts(2 * blk + half, 128)],
                                     b8[:, 2 * j:2 * j + 2, ds(ch * 512, 512)],
                                     start=(j == 0), stop=(j == T // 2 - 1), perf_mode=DR)
                nc.scalar.activation(out=p_t[:, half, :], in_=s_ps, func=AF.Exp,
                                     scale=SCALE / WS, bias=shb)
            if prev is not None:
                emit_av(prev[0], prev[1])
            prev = (blk, p_t)
            if ch > 0 and blk == 3:
                pe_epilogue(ch - 1)
        emit_av(prev[0], prev[1])

        # rowsum reciprocal (DVE) + o eviction; the eviction frees the o
        # accumulators for the next chunk's AV matmuls. |o| < ~100 in fp8.
        rsinv = small.tile([1, 512], F32, tag="rsinv", name=f"rsinv{ch}")
        nc.vector.reciprocal(rsinv, rs_ps)
        env[f"rsinv{ch}"] = rsinv
        for tc_in in range(T):
            if tc_in % 2 == 0:
                nc.scalar.activation(out=o8[:, tc_in, ds(ch * 512, 512)],
                                     in_=o_ps[tc_in], func=AF.Identity)
            else:
                nc.vector.tensor_copy(o8[:, tc_in, ds(ch * 512, 512)], o_ps[tc_in])

    pe_epilogue(NQC - 1)


_NC_CACHE = {}


def _get_nc(reps: int = 1):
    if reps not in _NC_CACHE:
        _NC_CACHE[reps] = build_nc(reps)
    return _NC_CACHE[reps]


def make_in_maps(x, gn_weight, gn_bias, qkv_weight, qkv_bias, proj_weight, proj_bias):
    x = np.asarray(x, np.float32)
    qkv_weight = np.asarray(qkv_weight, np.float32)
    proj_weight = np.asarray(proj_weight, np.float32)
    qkv_bias = np.asarray(qkv_bias, np.float32)
    proj_bias = np.asarray(proj_bias, np.float32)
    gn_weight = np.asarray(gn_weight, np.float32)
    gn_bias = np.asarray(gn_bias, np.float32)

    Wq, Wk, Wv = qkv_weight[0:C], qkv_weight[C:2 * C], qkv_weight[2 * C:3 * C]
    wqkT = np.ascontiguousarray((WS * (Wq.T @ Wk)).astype(E4))
    wvT = np.ascontiguousarray((WS * Wv.T).astype(E4))
    wpT = np.ascontiguousarray((WS * proj_weight.T).astype(E4))

    def cols(v):  # [C] -> [128, T]
        return np.ascontiguousarray(v.reshape(T, 128).T.astype(np.float32))

    bqkv = WS * (Wk.T @ qkv_bias[0:C])
    fbv = proj_weight @ qkv_bias[2 * C:3 * C] + proj_bias

    p_idx = np.arange(128)
    selred = np.zeros((128, T, GROUPS), np.float32)
    selbc = np.zeros((GROUPS, C), np.float32)
    for t in range(T):
        g = t * (128 // GSIZE) + p_idx // GSIZE
        selred[p_idx, t, g] = 1.0 / GSIZE
        selbc[g, t * 128 + p_idx] = 1.0

    shared = {
        "wqkT": wqkT, "wvT": wvT, "wpT": wpT,
        "bqk": cols(bqkv),
        "gnw": cols(gn_weight), "gnb": cols(gn_bias), "fb": cols(fbv),
        "selred": selred, "selbc": selbc,
    }
    in_maps = []
    for core in range(8):
        b, qb = core // 4, core % 4
        xb = x[b].reshape(C, N)
        xr = np.ascontiguousarray(np.roll(xb, -qb * NQ, axis=1))
        m = dict(shared)
        # chunk-major fp8 x: [128, NCH, T, 512], [p, nch, t, j] = xr[t*128+p, nch*512+j]
        m["x8"] = np.ascontiguousarray(
            xr.reshape(T, 128, NCH, 512).transpose(1, 2, 0, 3).astype(E4))
        m["xq"] = np.ascontiguousarray(xr[:, 0:NQ].astype(np.float16))
        in_maps.append(m)
    return in_maps


def kernel(x, gn_weight, gn_bias, qkv_weight, qkv_bias, proj_weight, proj_bias):
    nc = _get_nc(1)
    in_maps = make_in_maps(x, gn_weight, gn_bias, qkv_weight, qkv_bias,
                           proj_weight, proj_bias)
    res = run_bass_kernel_spmd(nc, in_maps, core_ids=list(range(8)))
    out = np.empty((B, C, N), np.float32)
    for core in range(8):
        b, qb = core // 4, core % 4
        out[b][:, qb * NQ:(qb + 1) * NQ] = res.results[core]["out"]
    return out.reshape(B, C, H, W, D)
